# revision 2
# baseline (speedup 1.0000x reference)
"""Trainium2 Bass kernel for nn_GraphPooler (segment mean/max pooling + MLP).

Computation (reference):
    mean/max-pool self_feats [2e6, 128] over 10000 contiguous 200-node graphs,
    concat -> [10000, 256], 3-layer MLP -> sigmoid -> [10000, 1].

Strategy (8 NeuronCores, data-parallel over graphs):
  - Each core handles 1280 graphs (256000 node rows, ~131 MB fp32 read).
    Cores 0-6 start at graph 1250*c; core 7 starts at 8720 so its 1280-graph
    window ends exactly at graph 10000 (overlapping outputs are discarded).
    All per-core inputs are contiguous views of self_feats.
  - Per 16-graph "period" (3200 nodes), the SWDGE DMA loads a [128, 25*128]
    tile, casting fp32->fp16 inline.  Column-major node mapping
    node = 25*p + j puts graph g on partitions [8g, 8g+8) exactly (200=8*25).
  - TensorE, per j-column (lhsT = the [128, 128] fp16 column):
      * two is_transpose matmuls (identity halves) scatter featsT into PSUM
        half-buffers (bank-aligned 64-col writes, 1 cyc/col for fp16),
      * one matmul against a constant 0/1 block indicator accumulates exact
        fp32 per-graph feature sums [128d, 16g] in PSUM.
  - VectorE: one batched reduce_max (axis=XY) per 8-graph half-buffer
    directly from PSUM -> maxT sbuf [128, G].
  - ScalarE: drains the sum PSUM into meanT (the /200 mean scale is folded
    into W0's mean-half on the host), then MLP bias+relu/sigmoid epilogue.
  - MLP: W0 as lhsT in two 128-row K-halves (mean-half @ meanT + max-half
    @ maxT accumulated in PSUM), W1 likewise over h1's halves, W2 -> [1, G],
    sigmoid, DMA out.

The harness calls kernel(**inputs) with the full unsharded inputs and
expects the full [10000, 1] fp32 output.
"""

import numpy as np

import concourse.bacc as bacc
import concourse.tile as tile
from concourse import mybir
from concourse.bass_utils import run_bass_kernel_spmd

F32 = mybir.dt.float32
F16 = mybir.dt.float16
AF = mybir.ActivationFunctionType
AX = mybir.AxisListType

NCORES = 8
N_GRAPHS = 10000
NPG = 200          # nodes per graph
D = 128
GP = 16            # graphs per period
JCOLS = 25         # node columns per period tile (200 = 8 partitions * 25)
PERIOD_NODES = GP * NPG  # 3200
G_CORE = 1280      # graphs computed per core (16-aligned)
N_PERIODS = G_CORE // GP  # 80
CORE_ROWS = G_CORE * NPG  # 256000

# graph offset of each core's 1280-graph window; core 7 is pulled back so the
# window ends at graph 10000.  kept output = local graphs [KEEP, KEEP+1250).
CORE_G0 = [1250 * c for c in range(7)] + [N_GRAPHS - G_CORE]
PER_CORE_OUT = N_GRAPHS // NCORES  # 1250


def build_program(reps: int = 1, debug_pooled: bool = False):
    """Build the SPMD Bass program (identical on all 8 cores).

    reps > 1 wraps the whole compute in a hardware For-loop so test harnesses
    can measure steady-state device time via wall-clock deltas.
    """
    nc = bacc.Bacc("TRN2", target_bir_lowering=False, num_devices=NCORES)
    pooled_dbg = (
        nc.dram_tensor("pooled_dbg", [128, 2 * G_CORE], F32, kind="ExternalOutput")
        if debug_pooled
        else None
    )

    feats = nc.dram_tensor("feats", [CORE_ROWS, D], F32, kind="ExternalInput")
    ident = nc.dram_tensor("ident", [128, 128], F16, kind="ExternalInput")
    ind = nc.dram_tensor("ind", [128, GP], F16, kind="ExternalInput")
    w0m = nc.dram_tensor("w0m", [128, 256], F32, kind="ExternalInput")
    w0x = nc.dram_tensor("w0x", [128, 256], F32, kind="ExternalInput")
    w1 = nc.dram_tensor("w1", [256, 128], F32, kind="ExternalInput")
    w2 = nc.dram_tensor("w2", [128, 1], F32, kind="ExternalInput")
    b0 = nc.dram_tensor("b0", [256], F32, kind="ExternalInput")
    b1 = nc.dram_tensor("b1", [128], F32, kind="ExternalInput")
    b2 = nc.dram_tensor("b2", [1], F32, kind="ExternalInput")
    y = nc.dram_tensor("y", [G_CORE], F32, kind="ExternalOutput")

    with tile.TileContext(nc) as tc:
        with tc.tile_pool(name="consts", bufs=1) as cpool:
            ident_s = cpool.tile([128, 128], F16)
            nc.sync.dma_start(ident_s[:], ident[:])
            ind_s = cpool.tile([128, GP], F16)
            nc.sync.dma_start(ind_s[:], ind[:])
            w0m_s = cpool.tile([128, 256], F32)
            nc.sync.dma_start(w0m_s[:], w0m[:])
            w0x_s = cpool.tile([128, 256], F32)
            nc.sync.dma_start(w0x_s[:], w0x[:])
            w1a_s = cpool.tile([128, 128], F32)
            nc.sync.dma_start(w1a_s[:], w1[0:128, :])
            w1b_s = cpool.tile([128, 128], F32)
            nc.sync.dma_start(w1b_s[:], w1[128:256, :])
            w2_s = cpool.tile([128, 1], F32)
            nc.sync.dma_start(w2_s[:], w2[:])
            b0_s = cpool.tile([128, 2], F32)
            nc.sync.dma_start(b0_s[:], b0[:].rearrange("(h p) -> p h", p=128))
            b1_s = cpool.tile([128, 1], F32)
            nc.sync.dma_start(b1_s[:], b1[:].rearrange("(p o) -> p o", o=1))
            b2_s = cpool.tile([1, 1], F32)
            nc.sync.dma_start(b2_s[:], b2[:].rearrange("(p o) -> p o", o=1))

            pooledT = cpool.tile([128, 2 * G_CORE], F32, tag="pooledT")
            meanT = pooledT[:, 0:G_CORE]
            maxT = pooledT[:, G_CORE : 2 * G_CORE]
            h1 = cpool.tile([128, 2 * G_CORE], F32, tag="h1")
            h2 = cpool.tile([128, G_CORE], F32, tag="h2")
            ysb = cpool.tile([1, G_CORE], F32, tag="ysb")

            def emit_body():
                with (
                    tc.tile_pool(name="chunks", bufs=4) as chunk_pool,
                    tc.tile_pool(name="phalf", bufs=3, space="PSUM") as half_pool,
                    tc.tile_pool(name="pmean", bufs=2, space="PSUM") as mean_pool,
                ):
                    for per in range(N_PERIODS):
                        chunk = chunk_pool.tile([128, JCOLS * D], F16, tag="chunk")
                        src = feats[
                            per * PERIOD_NODES : (per + 1) * PERIOD_NODES, :
                        ].rearrange("(p r) d -> p (r d)", p=128)
                        nc.gpsimd.dma_start(chunk[:], src)  # fp32->fp16 cast DMA

                        pmean = mean_pool.tile([128, GP], F32, tag="pmean")
                        for half in range(2):
                            ph = half_pool.tile([128, JCOLS * 64], F16, tag="ph")
                            for j in range(JCOLS):
                                nc.tensor.matmul(
                                    ph[:, j * 64 : (j + 1) * 64],
                                    lhsT=chunk[:, j * D : (j + 1) * D],
                                    rhs=ident_s[:, half * 64 : (half + 1) * 64],
                                    is_transpose=True,
                                )
                            rview = ph[:].rearrange(
                                "p (j g m) -> p g j m", j=JCOLS, g=8, m=8
                            )
                            col = per * GP + half * 8
                            nc.vector.reduce_max(
                                maxT[:, col : col + 8], rview, axis=AX.XY
                            )
                        for j in range(JCOLS):
                            nc.tensor.matmul(
                                pmean[:],
                                lhsT=chunk[:, j * D : (j + 1) * D],
                                rhs=ind_s[:],
                                start=(j == 0),
                                stop=(j == JCOLS - 1),
                                skip_group_check=True,
                            )
                        nc.scalar.copy(meanT[:, per * GP : (per + 1) * GP], pmean[:])

                # ---- MLP over all G_CORE graphs ----
                gchunks = [(g, min(512, G_CORE - g)) for g in range(0, G_CORE, 512)]
                with tc.tile_pool(name="pmlp", bufs=4, space="PSUM") as mlp_pool:
                    for h in range(2):
                        for g0, gn in gchunks:
                            pm = mlp_pool.tile([128, 512], F32, tag="pm")
                            nc.tensor.matmul(
                                pm[:, 0:gn],
                                lhsT=w0m_s[:, h * 128 : (h + 1) * 128],
                                rhs=meanT[:, g0 : g0 + gn],
                                start=True,
                                stop=False,
                                skip_group_check=True,
                            )
                            nc.tensor.matmul(
                                pm[:, 0:gn],
                                lhsT=w0x_s[:, h * 128 : (h + 1) * 128],
                                rhs=maxT[:, g0 : g0 + gn],
                                start=False,
                                stop=True,
                                skip_group_check=True,
                            )
                            nc.scalar.activation(
                                h1[:, h * G_CORE + g0 : h * G_CORE + g0 + gn],
                                pm[:, 0:gn],
                                AF.Relu,
                                bias=b0_s[:, h : h + 1],
                            )
                    for g0, gn in gchunks:
                        pm = mlp_pool.tile([128, 512], F32, tag="pm")
                        nc.tensor.matmul(
                            pm[:, 0:gn],
                            lhsT=w1a_s[:],
                            rhs=h1[:, g0 : g0 + gn],
                            start=True,
                            stop=False,
                            skip_group_check=True,
                        )
                        nc.tensor.matmul(
                            pm[:, 0:gn],
                            lhsT=w1b_s[:],
                            rhs=h1[:, G_CORE + g0 : G_CORE + g0 + gn],
                            start=False,
                            stop=True,
                            skip_group_check=True,
                        )
                        nc.scalar.activation(
                            h2[:, g0 : g0 + gn], pm[:, 0:gn], AF.Relu,
                            bias=b1_s[:],
                        )
                    for g0, gn in gchunks:
                        pm1 = mlp_pool.tile([1, 512], F32, tag="pm1")
                        nc.tensor.matmul(
                            pm1[:, 0:gn],
                            lhsT=w2_s[:],
                            rhs=h2[:, g0 : g0 + gn],
                            start=True,
                            stop=True,
                            skip_group_check=True,
                        )
                        nc.scalar.activation(
                            ysb[:, g0 : g0 + gn], pm1[:, 0:gn], AF.Sigmoid,
                            bias=b2_s[:],
                        )
                nc.sync.dma_start(y[:], ysb[:])
                if pooled_dbg is not None:
                    nc.sync.dma_start(pooled_dbg[:], pooledT[:])

            if reps == 1:
                emit_body()
            else:
                with tc.For_i(0, reps, 1):
                    emit_body()

    nc.finalize()
    return nc


def _host_constants(W0, b0, W1, b1, W2, b2, scale):
    """Host-side constant prep (fp32/fp16 numpy)."""
    ident = np.eye(128, dtype=np.float16)
    ind = np.zeros((128, GP), dtype=np.float16)
    for p in range(128):
        ind[p, p // 8] = 1.0
    w0m = (np.asarray(W0[0:D, :], dtype=np.float32) * scale).astype(np.float32)
    w0x = np.ascontiguousarray(np.asarray(W0[D : 2 * D, :], dtype=np.float32))
    return {
        "ident": ident,
        "ind": ind,
        "w0m": w0m,
        "w0x": w0x,
        "w1": np.ascontiguousarray(np.asarray(W1, dtype=np.float32)),
        "w2": np.ascontiguousarray(np.asarray(W2, dtype=np.float32)),
        "b0": np.ascontiguousarray(np.asarray(b0, dtype=np.float32)),
        "b1": np.ascontiguousarray(np.asarray(b1, dtype=np.float32)),
        "b2": np.ascontiguousarray(np.asarray(b2, dtype=np.float32)),
    }


_PROGRAM_CACHE: dict = {}


def _get_program(reps: int = 1):
    if reps not in _PROGRAM_CACHE:
        _PROGRAM_CACHE[reps] = build_program(reps)
    return _PROGRAM_CACHE[reps]


def _numpy_fallback(self_feats, graph_size, W0, b0, W1, b1, W2, b2):
    """Pure-numpy reference path for non-uniform graph sizes (never hit with
    the standard setup_inputs, which is uniform 200)."""
    sizes = np.asarray(graph_size, dtype=np.int64)
    G = sizes.shape[0]
    x = np.asarray(self_feats, dtype=np.float32)
    offs = np.concatenate([[0], np.cumsum(sizes)])
    mean_feats = np.empty((G, x.shape[1]), np.float32)
    max_feats = np.empty((G, x.shape[1]), np.float32)
    for g in range(G):
        seg = x[offs[g] : offs[g + 1]]
        mean_feats[g] = seg.mean(axis=0)
        max_feats[g] = seg.max(axis=0)
    pooled = np.concatenate([mean_feats, max_feats], axis=1)
    h = np.maximum(pooled @ np.asarray(W0, np.float32) + np.asarray(b0, np.float32), 0)
    h = np.maximum(h @ np.asarray(W1, np.float32) + np.asarray(b1, np.float32), 0)
    z = h @ np.asarray(W2, np.float32) + np.asarray(b2, np.float32)
    return (1.0 / (1.0 + np.exp(-z))).astype(np.float32)


def _make_in_maps(inputs):
    consts = _host_constants(
        inputs["W0"], inputs["b0"], inputs["W1"], inputs["b1"],
        inputs["W2"], inputs["b2"], 1.0 / NPG,
    )
    x = np.asarray(inputs["self_feats"], dtype=np.float32)
    in_maps = []
    for c in range(NCORES):
        r0 = CORE_G0[c] * NPG
        m = {"feats": x[r0 : r0 + CORE_ROWS, :]}
        m.update(consts)
        in_maps.append(m)
    return in_maps


def kernel(self_feats, graph_size, W0, b0, W1, b1, W2, b2):
    sizes = np.asarray(graph_size)
    x = np.asarray(self_feats, dtype=np.float32)
    if not (
        sizes.shape == (N_GRAPHS,)
        and np.all(sizes == NPG)
        and x.shape == (N_GRAPHS * NPG, D)
    ):
        return _numpy_fallback(self_feats, graph_size, W0, b0, W1, b1, W2, b2)

    in_maps = _make_in_maps({
        "self_feats": x, "W0": W0, "b0": b0, "W1": W1, "b1": b1,
        "W2": W2, "b2": b2,
    })

    nc = _get_program(1)
    res = run_bass_kernel_spmd(nc, in_maps, list(range(NCORES)))

    out = np.empty((N_GRAPHS, 1), dtype=np.float32)
    for c in range(NCORES):
        keep0 = 0 if c < 7 else (1250 * 7 - CORE_G0[7])
        yc = res.results[c]["y"]
        out[c * PER_CORE_OUT : (c + 1) * PER_CORE_OUT, 0] = yc[
            keep0 : keep0 + PER_CORE_OUT
        ]
    return out



# revision 3
# speedup vs baseline: 1.2636x; 1.2636x over previous
"""Trainium2 Bass kernel for nn_GraphPooler (segment mean/max pooling + MLP).

Computation (reference):
    mean/max-pool self_feats [2e6, 128] over 10000 contiguous 200-node graphs,
    concat -> [10000, 256], 3-layer MLP -> sigmoid -> [10000, 1].

Strategy (8 NeuronCores, data-parallel over graphs):
  - Each core handles 1280 graphs (256000 node rows, ~131 MB fp32 read).
    Cores 0-6 start at graph 1250*c; core 7 starts at 8720 so its 1280-graph
    window ends exactly at graph 10000 (overlapping outputs are discarded).
  - Per 64-graph "chunk" (12800 nodes), one SWDGE DMA loads a [128, 100*128]
    tile, casting fp32->fp16 inline (6.55 MB HBM read per transfer).  Node
    n = 100*p + r puts graph g on partitions {2g, 2g+1} (200 = 2*100), with
    each partition's 100 nodes inside a single graph.
  - VectorE: pairwise elementwise-max tree (8 scalar_tensor_tensor ops over
    contiguous fp16 128-col blocks) reduces the 100 nodes-per-partition to a
    per-partition partial max [128, 128d] in SBUF.
  - TensorE: 100 accumulating matmuls (lhsT = chunk j-column, rhs = 0/1
    2-partition->graph indicator) build exact fp32 per-graph feature sums
    [128d, 64g] in PSUM; one is_transpose matmul flips the partial max to
    [128d, 128p]; VectorE then reduce_maxes partition pairs -> [128d, 64g].
  - The 3-layer MLP runs per chunk (columns are independent graphs), fully
    overlapped with the streaming loop: W0 as lhsT in two 128-row K-halves
    (mean-half @ meanT + max-half @ maxT accumulated in PSUM; the /200 mean
    scale is folded into W0's mean-half on the host), W1 over h1's halves,
    W2 -> [1, 64], sigmoid -> ysb; one final DMA writes y.

The harness calls kernel(**inputs) with the full unsharded inputs and
expects the full [10000, 1] fp32 output.
"""

import numpy as np

import concourse.bacc as bacc
import concourse.tile as tile
from concourse import mybir
from concourse.bass_utils import run_bass_kernel_spmd

F32 = mybir.dt.float32
F16 = mybir.dt.float16
AF = mybir.ActivationFunctionType
AX = mybir.AxisListType

NCORES = 8
N_GRAPHS = 10000
NPG = 200          # nodes per graph
D = 128
GPC = 64           # graphs per chunk
NPP = 100          # nodes per partition per chunk (2 partitions per graph)
CHUNK_NODES = 128 * NPP  # 12800
G_CORE = 1280      # graphs computed per core
N_CHUNKS = G_CORE // GPC  # 20
CORE_ROWS = G_CORE * NPG  # 256000

# graph offset of each core's 1280-graph window; core 7 is pulled back so the
# window ends at graph 10000.  kept output = local graphs [KEEP, KEEP+1250).
CORE_G0 = [1250 * c for c in range(7)] + [N_GRAPHS - G_CORE]
PER_CORE_OUT = N_GRAPHS // NCORES  # 1250


def build_program(reps: int = 1):
    """Build the SPMD Bass program (identical on all 8 cores).

    reps > 1 wraps the whole compute in a hardware For-loop so test harnesses
    can measure steady-state device time via wall-clock deltas.
    """
    nc = bacc.Bacc("TRN2", target_bir_lowering=False, num_devices=NCORES)

    feats = nc.dram_tensor("feats", [CORE_ROWS, D], F32, kind="ExternalInput")
    ident = nc.dram_tensor("ident", [128, 128], F16, kind="ExternalInput")
    ind = nc.dram_tensor("ind", [128, GPC], F16, kind="ExternalInput")
    w0m = nc.dram_tensor("w0m", [128, 256], F32, kind="ExternalInput")
    w0x = nc.dram_tensor("w0x", [128, 256], F32, kind="ExternalInput")
    w1 = nc.dram_tensor("w1", [256, 128], F32, kind="ExternalInput")
    w2 = nc.dram_tensor("w2", [128, 1], F32, kind="ExternalInput")
    b0 = nc.dram_tensor("b0", [256], F32, kind="ExternalInput")
    b1 = nc.dram_tensor("b1", [128], F32, kind="ExternalInput")
    b2 = nc.dram_tensor("b2", [1], F32, kind="ExternalInput")
    y = nc.dram_tensor("y", [G_CORE], F32, kind="ExternalOutput")

    MU = mybir.AluOpType.mult
    MX = mybir.AluOpType.max

    with tile.TileContext(nc) as tc:
        with tc.tile_pool(name="consts", bufs=1) as cpool:
            ident_s = cpool.tile([128, 128], F16)
            nc.sync.dma_start(ident_s[:], ident[:])
            ind_s = cpool.tile([128, GPC], F16)
            nc.sync.dma_start(ind_s[:], ind[:])
            w0m_s = cpool.tile([128, 256], F32)
            nc.sync.dma_start(w0m_s[:], w0m[:])
            w0x_s = cpool.tile([128, 256], F32)
            nc.sync.dma_start(w0x_s[:], w0x[:])
            w1a_s = cpool.tile([128, 128], F32)
            nc.sync.dma_start(w1a_s[:], w1[0:128, :])
            w1b_s = cpool.tile([128, 128], F32)
            nc.sync.dma_start(w1b_s[:], w1[128:256, :])
            w2_s = cpool.tile([128, 1], F32)
            nc.sync.dma_start(w2_s[:], w2[:])
            b0_s = cpool.tile([128, 2], F32)
            nc.sync.dma_start(b0_s[:], b0[:].rearrange("(h p) -> p h", p=128))
            b1_s = cpool.tile([128, 1], F32)
            nc.sync.dma_start(b1_s[:], b1[:].rearrange("(p o) -> p o", o=1))
            b2_s = cpool.tile([1, 1], F32)
            nc.sync.dma_start(b2_s[:], b2[:].rearrange("(p o) -> p o", o=1))

            # shared scratch for the DVE max tree (trees are serial on DVE, so
            # one buffer suffices; Tile serializes chunk-to-chunk reuse).
            S = cpool.tile([128, 75 * D], F16, tag="tree_scratch")
            ysb = cpool.tile([1, G_CORE], F32, tag="ysb")

            def emit_body():
                with (
                    tc.tile_pool(name="chunks", bufs=4) as chunk_pool,
                    tc.tile_pool(name="pmaxs", bufs=3) as pmax_pool,
                    tc.tile_pool(name="pooled", bufs=3) as pooled_pool,
                    tc.tile_pool(name="hid", bufs=2) as h_pool,
                    tc.tile_pool(name="pmean", bufs=2, space="PSUM") as mean_pool,
                    tc.tile_pool(name="ptp", bufs=2, space="PSUM") as tp_pool,
                    tc.tile_pool(name="pmlp", bufs=3, space="PSUM") as mlp_pool,
                    tc.tile_pool(name="pout", bufs=1, space="PSUM") as out_pool,
                ):
                    def emit_load(c):
                        chunk = chunk_pool.tile([128, CHUNK_NODES], F16, tag="chunk")
                        src = feats[
                            c * CHUNK_NODES : (c + 1) * CHUNK_NODES, :
                        ].rearrange("(p r) d -> p (r d)", p=128)
                        nc.gpsimd.dma_start(chunk[:], src)  # fp32->fp16 cast DMA
                        return chunk

                    def emit_tree(chunk):
                        # pairwise max over the 100 node-blocks per partition;
                        # every op reads/writes contiguous fp16 column ranges.
                        pmax = pmax_pool.tile([128, 128], F16, tag="pmax")
                        stt = nc.vector.scalar_tensor_tensor

                        def mx(dst, a, b):
                            stt(dst, a, 1.0, b, MU, MX)

                        mx(S[:, 0 : 50 * D], chunk[:, 0 : 50 * D], chunk[:, 50 * D : 100 * D])
                        mx(S[:, 50 * D : 75 * D], S[:, 0 : 25 * D], S[:, 25 * D : 50 * D])
                        mx(S[:, 0 : 12 * D], S[:, 50 * D : 62 * D], S[:, 62 * D : 74 * D])
                        mx(S[:, 12 * D : 18 * D], S[:, 0 : 6 * D], S[:, 6 * D : 12 * D])
                        mx(S[:, 18 * D : 21 * D], S[:, 12 * D : 15 * D], S[:, 15 * D : 18 * D])
                        mx(S[:, 21 * D : 22 * D], S[:, 18 * D : 19 * D], S[:, 19 * D : 20 * D])
                        mx(S[:, 22 * D : 23 * D], S[:, 21 * D : 22 * D], S[:, 20 * D : 21 * D])
                        mx(pmax[:], S[:, 22 * D : 23 * D], S[:, 74 * D : 75 * D])
                        return pmax

                    def emit_sums(chunk):
                        pmean = mean_pool.tile([128, GPC], F32, tag="pmean")
                        for j in range(NPP):
                            nc.tensor.matmul(
                                pmean[:],
                                lhsT=chunk[:, j * D : (j + 1) * D],
                                rhs=ind_s[:],
                                start=(j == 0),
                                stop=(j == NPP - 1),
                                skip_group_check=True,
                            )
                        meanT_c = pooled_pool.tile([128, GPC], F32, tag="meanT")
                        nc.scalar.copy(meanT_c[:], pmean[:])
                        return meanT_c

                    def emit_finish(c, pmax, meanT_c):
                        pmaxT = tp_pool.tile([128, 128], F16, tag="pmaxT")
                        nc.tensor.matmul(
                            pmaxT[:], lhsT=pmax[:], rhs=ident_s[:], is_transpose=True
                        )
                        maxT_c = pooled_pool.tile([128, GPC], F32, tag="maxT")
                        rview = pmaxT[:].rearrange("p (g m) -> p g m", g=GPC, m=2)
                        nc.vector.reduce_max(maxT_c[:], rview, axis=AX.X)

                        h1_c = h_pool.tile([128, 2 * GPC], F32, tag="h1")
                        for h in range(2):
                            pm = mlp_pool.tile([128, GPC], F32, tag="pm")
                            nc.tensor.matmul(
                                pm[:],
                                lhsT=w0m_s[:, h * 128 : (h + 1) * 128],
                                rhs=meanT_c[:],
                                start=True,
                                stop=False,
                                skip_group_check=True,
                            )
                            nc.tensor.matmul(
                                pm[:],
                                lhsT=w0x_s[:, h * 128 : (h + 1) * 128],
                                rhs=maxT_c[:],
                                start=False,
                                stop=True,
                                skip_group_check=True,
                            )
                            nc.scalar.activation(
                                h1_c[:, h * GPC : (h + 1) * GPC], pm[:], AF.Relu,
                                bias=b0_s[:, h : h + 1],
                            )
                        pm = mlp_pool.tile([128, GPC], F32, tag="pm")
                        nc.tensor.matmul(
                            pm[:], lhsT=w1a_s[:], rhs=h1_c[:, 0:GPC],
                            start=True, stop=False, skip_group_check=True,
                        )
                        nc.tensor.matmul(
                            pm[:], lhsT=w1b_s[:], rhs=h1_c[:, GPC : 2 * GPC],
                            start=False, stop=True, skip_group_check=True,
                        )
                        h2_c = h_pool.tile([128, GPC], F32, tag="h2")
                        nc.scalar.activation(h2_c[:], pm[:], AF.Relu, bias=b1_s[:])
                        pm1 = out_pool.tile([1, GPC], F32, tag="pm1")
                        nc.tensor.matmul(
                            pm1[:], lhsT=w2_s[:], rhs=h2_c[:],
                            start=True, stop=True, skip_group_check=True,
                        )
                        nc.scalar.activation(
                            ysb[:, c * GPC : (c + 1) * GPC], pm1[:], AF.Sigmoid,
                            bias=b2_s[:],
                        )

                    prev = None
                    for c in range(N_CHUNKS):
                        chunk = emit_load(c)
                        if prev is not None:
                            emit_finish(*prev)
                        pmax = emit_tree(chunk)
                        meanT_c = emit_sums(chunk)
                        prev = (c, pmax, meanT_c)
                    emit_finish(*prev)
                nc.sync.dma_start(y[:], ysb[:])

            if reps == 1:
                emit_body()
            else:
                with tc.For_i(0, reps, 1):
                    emit_body()

    nc.finalize()
    return nc


def _host_constants(W0, b0, W1, b1, W2, b2, scale):
    """Host-side constant prep (fp32/fp16 numpy)."""
    ident = np.eye(128, dtype=np.float16)
    ind = np.zeros((128, GPC), dtype=np.float16)
    for p in range(128):
        ind[p, p // 2] = 1.0
    w0m = (np.asarray(W0[0:D, :], dtype=np.float32) * scale).astype(np.float32)
    w0x = np.ascontiguousarray(np.asarray(W0[D : 2 * D, :], dtype=np.float32))
    return {
        "ident": ident,
        "ind": ind,
        "w0m": w0m,
        "w0x": w0x,
        "w1": np.ascontiguousarray(np.asarray(W1, dtype=np.float32)),
        "w2": np.ascontiguousarray(np.asarray(W2, dtype=np.float32)),
        "b0": np.ascontiguousarray(np.asarray(b0, dtype=np.float32)),
        "b1": np.ascontiguousarray(np.asarray(b1, dtype=np.float32)),
        "b2": np.ascontiguousarray(np.asarray(b2, dtype=np.float32)),
    }


_PROGRAM_CACHE: dict = {}


def _get_program(reps: int = 1):
    if reps not in _PROGRAM_CACHE:
        _PROGRAM_CACHE[reps] = build_program(reps)
    return _PROGRAM_CACHE[reps]


def _numpy_fallback(self_feats, graph_size, W0, b0, W1, b1, W2, b2):
    """Pure-numpy reference path for non-uniform graph sizes (never hit with
    the standard setup_inputs, which is uniform 200)."""
    sizes = np.asarray(graph_size, dtype=np.int64)
    G = sizes.shape[0]
    x = np.asarray(self_feats, dtype=np.float32)
    offs = np.concatenate([[0], np.cumsum(sizes)])
    mean_feats = np.empty((G, x.shape[1]), np.float32)
    max_feats = np.empty((G, x.shape[1]), np.float32)
    for g in range(G):
        seg = x[offs[g] : offs[g + 1]]
        mean_feats[g] = seg.mean(axis=0)
        max_feats[g] = seg.max(axis=0)
    pooled = np.concatenate([mean_feats, max_feats], axis=1)
    h = np.maximum(pooled @ np.asarray(W0, np.float32) + np.asarray(b0, np.float32), 0)
    h = np.maximum(h @ np.asarray(W1, np.float32) + np.asarray(b1, np.float32), 0)
    z = h @ np.asarray(W2, np.float32) + np.asarray(b2, np.float32)
    return (1.0 / (1.0 + np.exp(-z))).astype(np.float32)


def _make_in_maps(inputs):
    consts = _host_constants(
        inputs["W0"], inputs["b0"], inputs["W1"], inputs["b1"],
        inputs["W2"], inputs["b2"], 1.0 / NPG,
    )
    x = np.asarray(inputs["self_feats"], dtype=np.float32)
    in_maps = []
    for c in range(NCORES):
        r0 = CORE_G0[c] * NPG
        m = {"feats": x[r0 : r0 + CORE_ROWS, :]}
        m.update(consts)
        in_maps.append(m)
    return in_maps


def kernel(self_feats, graph_size, W0, b0, W1, b1, W2, b2):
    sizes = np.asarray(graph_size)
    x = np.asarray(self_feats, dtype=np.float32)
    if not (
        sizes.shape == (N_GRAPHS,)
        and np.all(sizes == NPG)
        and x.shape == (N_GRAPHS * NPG, D)
    ):
        return _numpy_fallback(self_feats, graph_size, W0, b0, W1, b1, W2, b2)

    in_maps = _make_in_maps({
        "self_feats": x, "W0": W0, "b0": b0, "W1": W1, "b1": b1,
        "W2": W2, "b2": b2,
    })

    nc = _get_program(1)
    res = run_bass_kernel_spmd(nc, in_maps, list(range(NCORES)))

    out = np.empty((N_GRAPHS, 1), dtype=np.float32)
    for c in range(NCORES):
        keep0 = 0 if c < 7 else (1250 * 7 - CORE_G0[7])
        yc = res.results[c]["y"]
        out[c * PER_CORE_OUT : (c + 1) * PER_CORE_OUT, 0] = yc[
            keep0 : keep0 + PER_CORE_OUT
        ]
    return out
